# revision 60
# baseline (speedup 1.0000x reference)
"""GAT message-passing kernel for Trainium2 (8 NeuronCores, data-parallel over batch).

Math (per batch element b, derived from the reference nn.Module):
    x      = nodes.reshape(N, D)
    self_e = mlp2(x, self_*)                 # [N, H]
    nb_e   = mlp2(x, nb_*)                   # [N, H]
    U      = self_e @ comb_w1[:H]            # [N, H]  (i side)
    V      = nb_e @ comb_w1[H:] + comb_b1    # [N, H]  (j side)
    scores(i,j) = leaky(U_i + V_j) @ w2 + b2
                = 0.8*relu(U_i+V_j)@w2 + 0.2*(sU_i + sV_j) + const_i
    Softmax over j is invariant to per-i constants, so only
      s'(i,j) = 0.8*relu(U_i+V_j)@w2 + 0.2*sV_j  matters, and
      exp(s') factorizes as exp(0.8 relu(...)@w2) * exp(0.2 sV_j).
    E^T[j,i] = edges[j,i]*(j!=i)*exp(0.2 sV_j) * exp(0.8 relu(U_i+V_j)@w2)
    denom[i] = sum_j E^T[j,i]; gate = denom > eps; recip = gate/denom
    out[i]   = gate * (recip * (E^T)^T @ nb_e + self_e)
    (|scores| < 2, so exp needs no max-subtraction.)

Device mapping (one core per batch element; ~42.6us/core in the CoreSim cost
model vs 76263ns baseline, elementwise-build-bound). The pairwise stage uses the transposed
(g,h)-on-partitions layout: partitions = (i-parity g, h), free = j, so one
tensor_scalar(add,max)/activation(Relu,bias) op builds relu(V + U_i) for TWO
i's at once as a [128, 512] bf16 tile; PAIR_PATTERN splits the 256 builds
across DVE (4x perf mode, ~194ns) / Pool (~450ns) / ACT (~612ns) so all three
engines finish together.

Scores are computed J-MAJOR with the relu tile as the STATIONARY matmul
operand: per pair, 4 tiny matmuls (lhsT = a [128,128] j-chunk of the relu
tile, rhs = the two 0.8*w2 columns) write scores^T[j, 2p:2p+2] into one of 4
per-j-tile PSUM banks. Ldweights is free and each matmul is only 2 output
rows, so PE drops to ~8% busy, the [i,j]->[j,i] ET transposes disappear, and
exp(score + 0.2 sV_j) reads the banks directly with the per-j sV as the ACT
bias (per-matmul start/stop groups keep the banks readable mid-stream). The
softmax denominator rides along as a ones-column appended to nb_e in the
aggregation matmuls. Mask tiles are binary edges^T * (1-eye).

Precompute: host sends x^T (+ the first-stage weights in the same DMA),
host-folds w2@comb_w1 so U2/Vrep come straight from h1 (eT_n/eT_s are
produced off the critical path for selfe/nbe), fp32r matmuls (1 cyc/row at
>=256 free), zero-bias fast path when all biases are zero (the harness case),
and a dummy activation up front hides the ACT table load. The main loop
software-pipelines post-stages (exp/ET/agg/assembly) between build emissions
at i-half/quarter granularity.
"""

import os
import sys

sys.path.insert(0, "/opt/trn_rl_repo")

import numpy as np
import ml_dtypes

import concourse.bass as bass
import concourse.bacc as bacc
import concourse.tile as tile
from concourse import mybir, bass2jax
from concourse.bass_utils import run_bass_kernel_spmd

B, N, H, D = 8, 512, 64, 128
NCORES = 8
NT = N // 128          # 4 i/j tiles of 128
NPAIR = N // 2         # 256 i-pairs
F32 = mybir.dt.float32
F32R = mybir.dt.float32r
BF16 = mybir.dt.bfloat16
U8 = mybir.dt.uint8

# Per-pair build-engine assignment: 'v' DVE (4x bf16, ~194ns), 'a' ACT
# (~612ns), 'p' Pool (~450ns). Counts tuned so the three engines finish
# together; the last few pairs are forced onto DVE so the tail exp isn't
# gated by a slow build.
def _mk_pattern(nv, na, np_):
    share = {"v": nv, "a": na, "p": np_}
    tot = nv + na + np_
    credit = {k: 0.0 for k in share}
    out = []
    for _ in range(tot):
        for k in share:
            credit[k] += share[k] / tot
        c = max(credit, key=credit.get)
        out.append(c)
        credit[c] -= 1.0
    return "".join(out)


def _full_pattern(cycle, npair=256, vtail=6):
    pat = [cycle[p % len(cycle)] for p in range(npair)]
    for p in range(npair - vtail, npair):
        if pat[p] != "v":
            # swap with the latest earlier 'v'
            for q in range(npair - vtail - 1, -1, -1):
                if pat[q] == "v":
                    pat[q], pat[p] = pat[p], "v"
                    break
    return "".join(pat)


PAIR_PATTERN = _full_pattern(
    os.environ.get("GAT_PAIR_PATTERN", _mk_pattern(41, 11, 19)))

_CACHE = {}


def _build_module(zero_bias=True):
    nc = bacc.Bacc("TRN2", target_bir_lowering=False, debug=False, num_devices=NCORES)

    # nodes = [x^T | self_w1 | nb_w1] merged on the host: one DMA covers the
    # data and the first-stage weights
    nodes = nc.dram_tensor("nodes", [D, N + 128], F32R, kind="ExternalInput")
    edges = nc.dram_tensor("edges", [N, N], U8, kind="ExternalInput")
    wpackB = nc.dram_tensor("wpackB", [64, 128], F32R, kind="ExternalInput")
    wpackC = nc.dram_tensor("wpackC", [128, 128], F32, kind="ExternalInput")
    bvec = nc.dram_tensor("bvec", [64, 7], F32, kind="ExternalInput")
    bfpack = nc.dram_tensor("bfpack", [128, 388], BF16, kind="ExternalInput")

    out = nc.dram_tensor("out", [N, H], F32, kind="ExternalOutput")

    with tile.TileContext(nc) as tc:
        _emit(nc, tc, locals())
    nc.compile()
    return nc


def _emit(nc, tc, t):
    AF = mybir.ActivationFunctionType
    OP = mybir.AluOpType
    PAT = PAIR_PATTERN
    assert all(c in "vap" for c in PAT), PAT

    zero_bias = t["zero_bias"]

    with (
        tc.tile_pool(name="persist", bufs=1) as P,
        tc.tile_pool(name="ework", bufs=3) as EW,
        tc.tile_pool(name="edges", bufs=4) as EB,
        tc.tile_pool(name="reluv", bufs=13) as RLV,
        tc.tile_pool(name="relua", bufs=5) as RLA,
        tc.tile_pool(name="relup", bufs=5) as RLP,
        tc.tile_pool(name="xexp", bufs=3) as XE,
        tc.tile_pool(name="small", bufs=4) as SM,
        tc.tile_pool(name="psumS", bufs=1, space="PSUM") as SC,
        tc.tile_pool(name="psumT", bufs=1, space="PSUM") as PT,
        tc.tile_pool(name="psumM", bufs=2, space="PSUM") as PM,
        tc.tile_pool(name="psumA", bufs=1, space="PSUM") as PA,
    ):
        RLOF = {"v": RLV, "a": RLA, "p": RLP}
        # ---------- input DMAs (small, need-ordered; all on the SP queue) ---
        # nodes arrive pre-transposed [D, N] from the host.
        xw = P.tile([D, N + 128], F32R, tag="xw")
        nc.sync.dma_start(out=xw[:, 0:384], in_=t["nodes"].ap()[:, 0:384])
        nc.sync.dma_start(out=xw[:, 384:640], in_=t["nodes"].ap()[:, 384:640])
        xT = xw[:, 128:640]
        wpB = P.tile([64, 128], F32R, tag="wpackB")
        nc.sync.dma_start(out=wpB[:], in_=t["wpackB"].ap())
        bp = P.tile([128, 388], BF16, tag="bfpack")
        nc.sync.dma_start(out=bp[:], in_=t["bfpack"].ap())
        if not zero_bias:
            bv = P.tile([64, 7], F32, tag="bvec")
            nc.sync.dma_start(out=bv[:], in_=t["bvec"].ap())
        esb_all = EB.tile([128, NT, N], U8, tag="edges_in", name="esb_all")
        nc.sync.dma_start(out=esb_all[:],
                          in_=t["edges"].ap().rearrange("(t p) j -> p t j", t=NT))
        esbs = [esb_all[:, jt, :] for jt in range(NT)]
        wpC = P.tile([128, 128], F32, tag="wpackC")
        nc.sync.dma_start(out=wpC[:], in_=t["wpackC"].ap())

        # early dummy activation so the ACT table load (~1.3us) happens while
        # the DMAs are still in flight, off the first real activation
        warm = SM.tile([1, 1], F32, tag="warm", name="warm")
        nc.gpsimd.memset(warm[:], 0.0)
        warm2 = SM.tile([1, 1], F32, tag="warm2", name="warm2")
        nc.scalar.activation(out=warm2[:], in_=warm[:], func=AF.Identity, scale=1.0)

        # ---------- constant views ----------
        w1s, w1n = xw[:, 0:64], xw[:, 64:128]
        w2s, wfs = wpB[:, 0:64], wpB[:, 64:128]
        w2n, wfn = bp[0:64, 258:322], bp[0:64, 322:386]
        w2cb = bp[0:64, 386:387]
        if not zero_bias:
            b1s, b1n = bv[:, 0:1], bv[:, 1:2]
            b2sc, b2nc = bv[:, 2:3], bv[:, 3:4]
            ubias, vbias = bv[:, 5:6], bv[:, 6:7]
        ive = bp[:, 128:256]
        w2pair = bp[:, 256:258]     # [128, 2]: col0 = 0.8*w2 on g0, col1 on g1

        # ---------- tiny MLPs, chunked by 256 cols (h on partitions) --------
        # nb chain in bf16, self chain in f32r (self_e adds into the output).
        # zero_bias (the harness case): skip the +b1 stage and apply leaky
        # straight from PSUM; otherwise go through a bias activation first.
        h1T_n = P.tile([H, N], BF16, tag="h1T_n")
        h1T_s = P.tile([H, N], F32R, tag="h1T_s")
        eT_n = P.tile([H, N], BF16, tag="eT_n")
        eT_s = P.tile([H, N], F32, tag="eT_s")
        Vrep = P.tile([128, N], BF16, tag="Vrep")
        U2 = P.tile([128, NPAIR], F32, tag="U2")

        for k in range(2):
            cs = bass.ts(k, 256)
            pm = PM.tile([64, 256], F32, tag="pm", name="pm_w1n")
            nc.tensor.matmul(pm[:], w1n, xT[:, cs], start=True, stop=True)
            zn = EW.tile([H, 256], BF16, tag="zn", name="zn")
            if zero_bias:
                nc.scalar.activation(out=zn[:], in_=pm[:], func=AF.Identity,
                                     scale=1.0)
            else:
                nc.scalar.activation(out=zn[:], in_=pm[:], func=AF.Identity,
                                     bias=b1n, scale=1.0)
            nc.vector.scalar_tensor_tensor(out=h1T_n[:, cs], in0=zn[:],
                                           scalar=0.2, in1=zn[:],
                                           op0=OP.mult, op1=OP.max)
            pm = PM.tile([64, 256], F32, tag="pm", name="pm_w1s")
            nc.tensor.matmul(pm[:], w1s, xT[:, cs], start=True, stop=True)
            zs = EW.tile([H, 256], F32, tag="zs", name="zs")
            if zero_bias:
                nc.scalar.activation(out=zs[:], in_=pm[:], func=AF.Identity,
                                     scale=1.0)
            else:
                nc.scalar.activation(out=zs[:], in_=pm[:], func=AF.Identity,
                                     bias=b1s, scale=1.0)
            nc.vector.scalar_tensor_tensor(out=h1T_s[:, cs], in0=zs[:],
                                           scalar=0.2, in1=zs[:],
                                           op0=OP.mult, op1=OP.max)

        # U/V via host-folded weights (skips the eT stage on the critical
        # path; eT_n/eT_s are still produced later for selfe/nbe)
        for k in range(2):
            cs = bass.ts(k, 256)
            pm = PM.tile([64, 256], F32, tag="pm", name="pm_wfn")
            nc.tensor.matmul(pm[:], wfn, h1T_n[:, cs], start=True, stop=True)
            if zero_bias:
                nc.scalar.activation(out=Vrep[0:64, cs], in_=pm[:],
                                     func=AF.Identity, scale=1.0)
                nc.vector.tensor_copy(out=Vrep[64:128, cs], in_=pm[:])
            else:
                nc.scalar.activation(out=Vrep[0:64, cs], in_=pm[:],
                                     func=AF.Identity, bias=vbias, scale=1.0)
                nc.vector.tensor_scalar_add(out=Vrep[64:128, cs], in0=pm[:],
                                            scalar1=vbias)
            pm = PM.tile([64, 256], F32, tag="pm", name="pm_wfs")
            nc.tensor.matmul(pm[:], wfs, h1T_s[:, cs], start=True, stop=True)
            psplit = pm[:].rearrange("p (i g) -> p i g", g=2)
            if zero_bias:
                nc.vector.tensor_copy(out=U2[0:64, bass.ts(k, 128)],
                                      in_=psplit[:, :, 0])
                nc.scalar.activation(out=U2[64:128, bass.ts(k, 128)],
                                     in_=psplit[:, :, 1], func=AF.Identity,
                                     scale=1.0)
            else:
                nc.vector.tensor_scalar_add(out=U2[0:64, bass.ts(k, 128)],
                                            in0=psplit[:, :, 0], scalar1=ubias)
                nc.scalar.activation(out=U2[64:128, bass.ts(k, 128)],
                                     in_=psplit[:, :, 1], func=AF.Identity,
                                     bias=ubias, scale=1.0)

        # ---------- deferred precompute: selfe / nbe+ones / 0.2*sV ----------
        # Emitted a few pairs into the main loop so the Pool/PE queues start
        # on builds first; only needed by the post stages (~halfway in).
        selfe, nbe_aug = [], []
        svT = P.tile([128, NT], F32, tag="svT")

        def emit_precomp2():
            for k in range(2):
                cs = bass.ts(k, 256)
                pm = PM.tile([64, 256], F32, tag="pm", name="pm_w2n")
                nc.tensor.matmul(pm[:], w2n, h1T_n[:, cs], start=True, stop=True)
                if zero_bias:
                    nc.scalar.activation(out=eT_n[:, cs], in_=pm[:],
                                         func=AF.Identity, scale=1.0)
                else:
                    nc.scalar.activation(out=eT_n[:, cs], in_=pm[:],
                                         func=AF.Identity, bias=b2nc, scale=1.0)
                pm = PM.tile([64, 256], F32, tag="pm", name="pm_w2s")
                nc.tensor.matmul(pm[:], w2s, h1T_s[:, cs], start=True, stop=True)
                if zero_bias:
                    nc.vector.tensor_copy(out=eT_s[:, cs], in_=pm[:])
                else:
                    nc.vector.tensor_scalar_add(out=eT_s[:, cs], in0=pm[:],
                                                scalar1=b2sc)
            for it in range(NT):
                pt = PT.tile([128, 128], F32, tag="pt", name="pts",
                             padded_shape=[128, 128])
                nc.tensor.transpose(pt[:, 0:64], eT_s[:, bass.ts(it, 128)],
                                    wpC[0:64, 0:64])
                se = P.tile([128, H], F32, tag=f"selfe{it}", name="se")
                nc.scalar.activation(out=se[:], in_=pt[:, 0:64],
                                     func=AF.Identity, scale=1.0)
                selfe.append(se)
                ptn = PT.tile([128, 128], BF16, tag="pt", name="ptn",
                              padded_shape=[128, 128])
                nc.tensor.transpose(ptn[:, 0:64], eT_n[:, bass.ts(it, 128)],
                                    bp[0:64, 0:64])
                # col 64 = 1.0: the agg matmul then also produces the softmax
                # denominator as output column 64 (no separate ones-matmuls).
                ne = P.tile([128, H + 1], BF16, tag=f"nbe{it}", name="ne")
                nc.vector.tensor_copy(out=ne[:, 0:64], in_=ptn[:, 0:64])
                nc.gpsimd.memset(ne[:, 64:65], 1.0)
                nbe_aug.append(ne)
            # 0.2*sV row -> [128, NT] per-partition (j) scalars, applied as
            # the exp bias so the mask tiles stay binary
            pm = PM.tile([64, 512], F32, tag="pm", name="pm_sv")
            nc.tensor.matmul(pm[:1, :], w2cb, Vrep[0:64, :], start=True, stop=True)
            sv_row = SM.tile([1, N], F32, tag="sv_row")
            nc.scalar.activation(out=sv_row[:], in_=pm[:1, :], func=AF.Identity,
                                 scale=0.2)
            pesv = PT.tile([128, 128], F32, tag="pt", name="pesv",
                           padded_shape=[128, 128])
            for tq in range(NT):
                nc.tensor.transpose(pesv[:, tq:tq + 1], sv_row[:, bass.ts(tq, 128)],
                                    wpC[0:1, 0:1])
            nc.vector.tensor_copy(out=svT[:], in_=pesv[:, 0:NT])

        # ---------- binary mask tiles: edges[j,i] * (j != i) ----------
        # (allocated here; built inside the main loop so DVE's build stream
        # is not delayed at the start)
        masks = [P.tile([128, N], BF16, tag=f"mask{jt}", name=f"mask{jt}")
                 for jt in range(NT)]

        def emit_mask(jt):
            mj = masks[jt]
            nc.gpsimd.tensor_copy(out=mj[:], in_=esbs[jt])
            nc.gpsimd.tensor_mul(out=mj[:, bass.ts(jt, 128)],
                                 in0=mj[:, bass.ts(jt, 128)], in1=ive[:])

        # ---------- main pass: j-major scores^T ----------
        # Pair p (i = 2p, 2p+1): its relu tile is the STATIONARY operand of 4
        # tiny matmuls (one per j-tile bank), rhs = the two 0.8*w2 columns ->
        # scores^T[j, 2p:2p+2] lands directly in [j, i] layout (no ET
        # transposes, denominators fused into agg).
        ET = [P.tile([128, N], BF16, tag=f"ET{jt}", name=f"ET{jt}") for jt in range(NT)]
        SCb = [SC.tile([128, N], F32, tag=f"sc{jt}", name=f"sc{jt}")
               for jt in range(NT)]
        pa_all = PA.tile([128, NT, H + 1], F32, tag="pa_all")

        def emit_pair(p):
            eng = PAT[p % len(PAT)]
            rl = RLOF[eng].tile([128, N], BF16, tag="relu")
            if eng == "v":
                nc.vector.tensor_scalar(out=rl[:], in0=Vrep[:],
                                        scalar1=U2[:, p:p + 1], scalar2=0.0,
                                        op0=OP.add, op1=OP.max)
            elif eng == "a":
                nc.scalar.activation(out=rl[:], in_=Vrep[:], func=AF.Relu,
                                     bias=U2[:, p:p + 1], scale=1.0)
            else:
                nc.gpsimd.tensor_scalar(out=rl[:], in0=Vrep[:],
                                        scalar1=U2[:, p:p + 1], scalar2=0.0,
                                        op0=OP.add, op1=OP.max)
            # disjoint 2-column slices: every matmul is its own psum group, so
            # the bank is never mid-group and exp can read finished columns
            for jt in range(NT):
                nc.tensor.matmul(SCb[jt][:, 2 * p:2 * p + 2],
                                 rl[:, bass.ts(jt, 128)], w2pair,
                                 start=True, stop=True)

        def emit_post_jt(its, jt):
            lo, hi = 128 * its[0], 128 * (its[-1] + 1)
            Xc = XE.tile([128, hi - lo], BF16, tag="X", name="Xc")
            nc.scalar.activation(out=Xc[:], in_=SCb[jt][:, lo:hi], func=AF.Exp,
                                 bias=svT[:, jt:jt + 1], scale=1.0)
            nc.gpsimd.tensor_mul(out=ET[jt][:, lo:hi], in0=Xc[:],
                                 in1=masks[jt][:, lo:hi])

        def emit_post_fin(its):
            for it in its:
                for jt in range(NT):
                    nc.tensor.matmul(pa_all[:, it, :], ET[jt][:, bass.ts(it, 128)],
                                     nbe_aug[jt][:], start=(jt == 0),
                                     stop=(jt == NT - 1))
                den = pa_all[:, it, H:H + 1]
                gate = SM.tile([128, 1], F32, tag="gate", name="gate")
                nc.vector.tensor_single_scalar(out=gate[:], in_=den,
                                               scalar=1e-6, op=OP.is_gt)
                dsafe = SM.tile([128, 1], F32, tag="dsafe", name="dsafe")
                nc.vector.tensor_scalar_max(out=dsafe[:], in0=den, scalar1=1e-30)
                recipg = SM.tile([128, 1], F32, tag="recipg", name="recipg")
                nc.vector.reciprocal(out=recipg[:], in_=dsafe[:])
                sg = SM.tile([128, H], F32, tag="sg")
                nc.gpsimd.tensor_scalar_mul(out=sg[:], in0=selfe[it][:],
                                            scalar1=gate[:])
                nc.gpsimd.tensor_mul(out=recipg[:], in0=recipg[:], in1=gate[:])
                ot = SM.tile([128, H], F32, tag="ot")
                nc.vector.scalar_tensor_tensor(out=ot[:], in0=pa_all[:, it, 0:H],
                                               scalar=recipg[:], in1=sg[:],
                                               op0=OP.mult, op1=OP.add)
                nc.sync.dma_start(out=t["out"].ap()[bass.ts(it, 128), :], in_=ot[:])

        POST1 = {150: 0, 155: 1, 160: 2, 165: 3}
        POST2 = {214: 0, 217: 1, 220: 2, 223: 3}
        for p in range(NPAIR):
            emit_pair(p)
            if p == 20:
                emit_precomp2()
            elif p == 48:
                emit_mask(0)
                emit_mask(1)
            elif p == 80:
                emit_mask(2)
                emit_mask(3)
            elif p in POST1:
                emit_post_jt([0, 1], POST1[p])
            elif p == 170:
                emit_post_fin([0, 1])
            elif p in POST2:
                emit_post_jt([2], POST2[p])
            elif p == 226:
                emit_post_fin([2])
        for jt in range(NT):
            emit_post_jt([3], jt)
        emit_post_fin([3])


def _host_constants(inputs):
    f32 = np.float32
    bf = ml_dtypes.bfloat16
    H_ = H
    w2 = np.asarray(inputs["comb_w2"], f32)[:, 0]      # [H]

    wpackA = np.zeros((128, 128), f32)
    wpackA[:, 0:64] = np.asarray(inputs["self_w1"], f32)
    wpackA[:, 64:128] = np.asarray(inputs["nb_w1"], f32)

    w2self = np.asarray(inputs["self_w2"], np.float64)
    w2nb = np.asarray(inputs["nb_w2"], np.float64)
    cw1 = np.asarray(inputs["comb_w1"], np.float64)
    wpackB = np.zeros((64, 128), f32)
    wpackB[:, 0:64] = w2self.astype(f32)
    wpackB[:, 64:128] = (w2self @ cw1[:H_]).astype(f32)      # wfold_s

    b2s = np.asarray(inputs["self_b2"], np.float64)
    b2n = np.asarray(inputs["nb_b2"], np.float64)
    bvec = np.stack([
        np.asarray(inputs["self_b1"], f32),
        np.asarray(inputs["nb_b1"], f32),
        np.asarray(inputs["self_b2"], f32),
        np.asarray(inputs["nb_b2"], f32),
        np.asarray(inputs["comb_b1"], f32),
        (cw1[:H_].T @ b2s).astype(f32),                      # ubias
        (cw1[H_:].T @ b2n
         + np.asarray(inputs["comb_b1"], np.float64)).astype(f32),  # vbias
    ], axis=1)

    bfpack = np.zeros((128, 388), f32)
    bfpack[:, 0:128] = np.eye(128, dtype=f32)
    bfpack[:, 128:256] = 1.0 - np.eye(128, dtype=f32)
    bfpack[0:64, 256] = 0.8 * w2
    bfpack[64:128, 257] = 0.8 * w2
    bfpack[0:64, 258:322] = w2nb.astype(f32)                 # w2n
    bfpack[0:64, 322:386] = (w2nb @ cw1[H_:]).astype(f32)    # wfold_n
    bfpack[0:64, 386] = w2                                   # w2cb

    return {
        "_w1pack": wpackA,          # merged into the per-core nodes DMA
        "wpackB": wpackB,
        "wpackC": np.eye(128, dtype=f32),
        "bvec": bvec,
        "bfpack": bfpack.astype(bf),
    }


def _build_fast_path(nc):
    """Cache a single jitted shard_map executable so repeat kernel() calls
    skip jax re-tracing (same lowering run_bass_kernel_spmd uses under axon)."""
    import jax
    from jax.sharding import Mesh, PartitionSpec
    from jax.experimental.shard_map import shard_map

    bass2jax.install_neuronx_cc_hook()
    pname = nc.partition_id_tensor.name if nc.partition_id_tensor else None
    in_names, out_names, out_avals = [], [], []
    for alloc in nc.m.functions[0].allocations:
        if not isinstance(alloc, mybir.MemoryLocationSet):
            continue
        name = alloc.memorylocations[0].name
        if alloc.kind == "ExternalInput":
            if name != pname:
                in_names.append(name)
        elif alloc.kind == "ExternalOutput":
            out_names.append(name)
            out_avals.append(jax.core.ShapedArray(tuple(alloc.tensor_shape),
                                                  mybir.dt.np(alloc.dtype)))
    all_names = in_names + out_names + ([pname] if pname else [])

    def _body(*args):
        operands = list(args)
        if pname is not None:
            operands.append(bass2jax.partition_id_tensor())
        return tuple(bass2jax._bass_exec_p.bind(
            *operands, out_avals=tuple(out_avals), in_names=tuple(all_names),
            out_names=tuple(out_names), lowering_input_output_aliases=(),
            sim_require_finite=True, sim_require_nnan=True, nc=nc))

    devices = jax.devices()[:NCORES]
    mesh = Mesh(np.asarray(devices), ("core",))
    n_io = len(in_names) + len(out_names)
    sharded = jax.jit(
        shard_map(_body, mesh=mesh, in_specs=(PartitionSpec("core"),) * n_io,
                  out_specs=(PartitionSpec("core"),) * len(out_names),
                  check_rep=False),
        keep_unused=True,
    )
    return sharded, in_names, out_names, out_avals


def kernel(**inputs):
    zb = all(not np.any(np.asarray(inputs[k]))
             for k in ("self_b1", "self_b2", "nb_b1", "nb_b2", "comb_b1"))
    first = ("nc", zb) not in _CACHE
    if first:
        _CACHE.clear()
        _CACHE[("nc", zb)] = _build_module(zero_bias=zb)
    nc = _CACHE[("nc", zb)]

    consts = _host_constants(inputs)
    w1pack = consts.pop("_w1pack")
    nodes = np.asarray(inputs["nodes"], np.float32).reshape(B, N, D)
    edges = (np.asarray(inputs["edges"]) != 0).astype(np.uint8)

    in_maps = []
    for c in range(NCORES):
        m = dict(consts)
        # pre-transposed [D, N] with the first-stage weights appended
        m["nodes"] = np.ascontiguousarray(
            np.concatenate([w1pack, nodes[c].T], axis=1))
        m["edges"] = edges[c]
        in_maps.append(m)

    if first:
        res = run_bass_kernel_spmd(nc, in_maps, core_ids=list(range(NCORES)))
        _CACHE["fast"] = _build_fast_path(nc)
        return np.stack([res.results[c]["out"] for c in range(NCORES)]).astype(np.float32)

    import jax
    sharded, in_names, out_names, out_avals = _CACHE["fast"]
    ckey = hash(tuple((k, v.tobytes()) for k, v in sorted(consts.items())))
    if _CACHE.get("ckey") != ckey:
        _CACHE["cdev"] = {
            n: jax.device_put(np.concatenate([np.asarray(in_maps[c][n])
                                              for c in range(NCORES)], axis=0))
            for n in in_names if n not in ("nodes", "edges")
        }
        _CACHE["zdev"] = [jax.device_put(np.zeros((NCORES * a.shape[0], *a.shape[1:]),
                                                  a.dtype)) for a in out_avals]
        _CACHE["ckey"] = ckey
    cdev = _CACHE["cdev"]
    concat_in = [cdev[n] if n in cdev else
                 np.concatenate([np.asarray(in_maps[c][n]) for c in range(NCORES)], axis=0)
                 for n in in_names]
    outs = sharded(*concat_in, *_CACHE["zdev"])
    i = out_names.index("out")
    return np.asarray(outs[i]).reshape(NCORES, N, H).astype(np.float32)
